# revision 44
# baseline (speedup 1.0000x reference)
"""MoE layer (top-2 of 8 experts, SwiGLU FFN) on 8 Trainium2 NeuronCores.

Strategy (per spec sharding_hint, expert-parallel):
  Launch 1 (data-parallel router): the 4096 tokens are sharded 512/core;
    each core computes its router logits tile in fp32 on the PE.
  Host dispatch: softmax/top-2/gates + per-expert token index lists are
    derived from the device logits (pure routing decisions + the tiny
    scalar loss epilogue).
  Launch 2 (expert-parallel FFN): core e holds expert e's w1/w3/w2 (bf16)
    and its gathered tokens (bf16); computes
    yT = (silu(x@w1) * (x@w3)) @ w2 * gate fully on-device.
  Host combine: scatter-add the two expert contributions per token.

All heavy math runs on-device; the host only routes/gathers/combines.
All DRAM inputs/outputs use partition-major tiled layouts ([128, ...]
with long contiguous per-partition runs) so every DMA descriptor moves
multi-KB and the transfers run at HBM line rate.
"""

import numpy as np
import ml_dtypes

import concourse.bass as bass
import concourse.tile as tile
from concourse import bacc, mybir
from concourse.bass_utils import run_bass_kernel_spmd

# Problem shapes (hardcoded per contract)
B, S, D, F, E = 2, 2048, 768, 2048, 8
N_TOK = B * S            # 4096
TOP_K = 2
AUX_COEF = 0.01
Z_COEF = 0.001
N_CORES = 8
P = 128                  # SBUF partitions
KD = D // P              # 6  k-tiles over D
KF = F // P              # 16 k-tiles over F
SHARD = N_TOK // N_CORES # 512 tokens/core in the router launch
N_CHUNKS = 3             # equal token chunks per expert in the FFN launch
# w1/w3 stream in f-tile-major pieces: small lead pieces let the PE start
# early, fat pieces amortize DMA fixed cost
W_PIECES = [1, 1, 1, 1, 1, 1, 1, 1, 8]
NH = 2                   # halves for w2 streaming

BF16 = mybir.dt.bfloat16
F32 = mybir.dt.float32

_router_cache = {}
_ffn_cache = {}

# Populated on every kernel() call; test harnesses may read these to report
# HW exec time when NTFF tracing is enabled (BASS_TRACE=1).
LAST_RESULTS = {}
LAST_EXEC_NS = None


def _pm(a, p=P):
    """[R, C] -> partition-major tiled [p, (R//p) * C], row r = t*p + q."""
    r, c = a.shape
    return np.ascontiguousarray(
        a.reshape(r // p, p, c).transpose(1, 0, 2).reshape(p, -1))


def _build_router_nc():
    """Data-parallel router: logitsT[E, tok] = router_w.T @ xT in fp32.

    Raw-block kernel (no Tile) to avoid the Tile exit-barrier cost.
    Transposed formulation: stationary = router_w tile [128, 8], moving =
    xT [128, 512] -> only KD=6 fat matmuls instead of 48 thin ones.
    """
    nc = bacc.Bacc("TRN2", target_bir_lowering=False, debug=False,
                   num_devices=N_CORES, enable_partition_id=False)
    # partition-major: xT_pm[p, k*SHARD + n] = x_shard[n, k*128+p]
    xT = nc.dram_tensor("xT", [P, KD * SHARD], F32, kind="ExternalInput").ap()
    rw = nc.dram_tensor("rw", [P, KD * E], F32, kind="ExternalInput").ap()
    logitsT = nc.dram_tensor("logitsT", [E, SHARD], F32,
                             kind="ExternalOutput").ap()

    xT_sb = nc.alloc_sbuf_tensor("xT_sb", [P, KD, SHARD], F32).ap()
    rw_sb = nc.alloc_sbuf_tensor("rw_sb", [P, KD, E], F32).ap()
    lg_sb = nc.alloc_sbuf_tensor("lg_sb", [E, SHARD], F32).ap()
    ps = nc.alloc_psum_tensor("ps_l", [E, SHARD], F32).ap()

    xT_r = xT.rearrange("p (t n) -> p t n", t=KD)
    rw_r = rw.rearrange("p (t e) -> p t e", t=KD)

    # DMAs on different HWDGE queues complete out of order, so each input
    # DMA gets its own semaphore (a shared counter would race).
    in_sems = [nc.alloc_semaphore(f"in_sem_{k}") for k in range(KD + 1)]

    ps_w = nc.alloc_psum_tensor("ps_warm", [P, P], F32).ap()

    with (
        nc.Block(no_gpsimd_drain=True) as block,
        nc.semaphore("dma_sem") as dma_sem,
        nc.semaphore("mm_sem") as mm_sem,
        nc.semaphore("cp_sem") as cp_sem,
    ):
        @block.sync
        def _(sync):
            sync.dma_start(rw_sb, rw_r).then_inc(in_sems[KD], 16)
            for k in range(0, KD, 2):
                sync.dma_start(xT_sb[:, k, :], xT_r[:, k, :]).then_inc(
                    in_sems[k], 16)
            sync.wait_ge(cp_sem, 1)
            sync.dma_start(logitsT, lg_sb).then_inc(dma_sem, 16)
            sync.wait_ge(dma_sem, 16)

        @block.scalar
        def _(scalar):
            # second HWDGE ring: odd k tiles in parallel with sync's evens
            for k in range(1, KD, 2):
                scalar.dma_start(xT_sb[:, k, :], xT_r[:, k, :]).then_inc(
                    in_sems[k], 16)

        @block.vector
        def _(vector):
            vector.wait_ge(mm_sem, 1)
            vector.tensor_copy(lg_sb, ps).then_inc(cp_sem, 1)

        @block.tensor
        def _(tensor):
            # warm the HAM clock-gate while the input DMAs stream: ~5us
            # of dummy PE activity (values are garbage, results unread)
            for _i in range(12):
                tensor.matmul(ps_w, lhsT=xT_sb[:, 0, 0:P],
                              rhs=xT_sb[:, 1, 0:P], start=True, stop=True)
            tensor.wait_ge(in_sems[KD], 16)
            for k in range(KD):
                tensor.wait_ge(in_sems[k], 16)
                mm = tensor.matmul(
                    ps,
                    lhsT=rw_sb[:, k, :],
                    rhs=xT_sb[:, k, :],
                    start=(k == 0),
                    stop=(k == KD - 1),
                )
                if k == KD - 1:
                    mm.then_inc(mm_sem, 1)

    nc.compile()
    return nc


def _build_ffn_nc(c_pad):
    """Expert-parallel SwiGLU FFN over gathered tokens.

    Partition-major DRAM layouts (ft = 128-wide f tile, h = f-half,
    c = chunk):
      xg   [N_CHUNKS, P, KD*clen]   bf16   xg[c][p][k*clen+j]  = x[tok, d]
      w1/3 [P, KF*KD*128]           bf16   [p][ft][k][j] = w[k*128+p, ft*128+j]
      w2   [NH, P, 8*768]           bf16   w2[h][p][i*768+d]   = w2[(h*8+i)*128+p, d]
      gate [1, c_pad]               f32    (partition-broadcast on load)
      y    [N_CHUNKS, P, KD*clen]   f32    y[c][p][k*clen+j]   = out[tok, d]
    """
    nc = bacc.Bacc("TRN2", target_bir_lowering=False, debug=False,
                   num_devices=N_CORES, enable_partition_id=False)
    clen = c_pad // N_CHUNKS
    assert clen * N_CHUNKS == c_pad and clen <= 512

    xg = nc.dram_tensor("xg", [N_CHUNKS, P, KD * clen], BF16,
                        kind="ExternalInput").ap()
    w1 = nc.dram_tensor("w1", [P, KF * KD * P], BF16,
                        kind="ExternalInput").ap()
    w3 = nc.dram_tensor("w3", [P, KF * KD * P], BF16,
                        kind="ExternalInput").ap()
    w2 = nc.dram_tensor("w2", [NH, P, (KF // NH) * D], BF16,
                        kind="ExternalInput").ap()
    gate = nc.dram_tensor("gate", [1, c_pad], F32, kind="ExternalInput").ap()
    y = nc.dram_tensor("y", [N_CHUNKS, P, KD * clen], F32,
                       kind="ExternalOutput").ap()

    KH = KF // NH         # 8

    with tile.TileContext(nc) as tc:
        with (
            tc.tile_pool(name="wsb", bufs=1) as wsb,
            tc.tile_pool(name="hsb", bufs=2) as hsb,
            tc.tile_pool(name="ysb", bufs=2) as ysb,
            tc.tile_pool(name="ps", bufs=2, space="PSUM") as ps,
        ):
            # Resident SBUF tensors.  Inputs stream over BOTH HWDGE rings
            # (sync + scalar) in PE consume-order; gate/output use the
            # SWDGE (gpsimd) path so they don't contend with the rings.
            xg_sb = wsb.tile([P, N_CHUNKS, KD, clen], BF16, tag="xg")
            w1_sb = wsb.tile([P, KF, KD, P], BF16, tag="w1")
            w3_sb = wsb.tile([P, KF, KD, P], BF16, tag="w3")
            w2_sb = wsb.tile([P, NH, KH, D], BF16, tag="w2")
            gate_sb = wsb.tile([P, c_pad], F32, tag="gate")

            # PE pre-warm: dummy matmuls on a zeroed tile keep the HAM
            # clock-gate busy while the first input DMAs stream in.
            warm_sb = hsb.tile([P, 512], BF16, tag="warm")
            nc.gpsimd.memset(warm_sb, 0)
            for _i in range(14):
                ps_w = ps.tile([P, 512], F32, tag="pswarm")
                nc.tensor.matmul(ps_w, lhsT=warm_sb[:, 0:P],
                                 rhs=warm_sb, start=True, stop=True)

            # three parallel DMA paths: w1 on the sync HWDGE ring, w3 on
            # the scalar HWDGE ring, activations/gate/output on SWDGE.
            # chunk-0 activations stream per k so the first matmul only
            # waits for one 90KB slice.
            xg0 = xg[0].rearrange("p (k j) -> p k j", k=KD)
            for k in range(KD):
                nc.gpsimd.dma_start(xg_sb[:, 0, k], xg0[:, k])
            w1_r = w1.rearrange("p (t k j) -> p t k j", t=KF, k=KD)
            w3_r = w3.rearrange("p (t k j) -> p t k j", t=KF, k=KD)
            ft0 = 0
            for npc in W_PIECES:
                fts = slice(ft0, ft0 + npc)
                nc.sync.dma_start(w1_sb[:, fts], w1_r[:, fts])
                nc.scalar.dma_start(w3_sb[:, fts], w3_r[:, fts])
                ft0 += npc
            for c in range(1, N_CHUNKS):
                nc.gpsimd.dma_start(
                    xg_sb[:, c], xg[c].rearrange("p (k j) -> p k j", k=KD))
            nc.sync.dma_start(
                w2_sb[:, 0], w2[0].rearrange("p (i d) -> p i d", i=KH))
            nc.scalar.dma_start(
                w2_sb[:, 1], w2[1].rearrange("p (i d) -> p i d", i=KH))

            gate_bcast = bass.AP(
                tensor=gate.tensor, offset=gate.offset,
                ap=[[0, P], gate.ap[1]],
            )
            nc.gpsimd.dma_start(gate_sb, gate_bcast)

            for c in range(N_CHUNKS):
                csl = slice(c * clen, (c + 1) * clen)
                # ---- up projections: hT[f, tok] = silu(w1.T x) * (w3.T x)
                h_sb = hsb.tile([P, KF, clen], BF16, tag="h")
                for ft in range(KF):
                    ps1 = ps.tile([P, clen], F32, tag="ps1")
                    for k in range(KD):
                        nc.tensor.matmul(
                            ps1,
                            lhsT=w1_sb[:, ft, k, :],
                            rhs=xg_sb[:, c, k, :],
                            start=(k == 0), stop=(k == KD - 1),
                        )
                    ps3 = ps.tile([P, clen], F32, tag="ps3")
                    for k in range(KD):
                        nc.tensor.matmul(
                            ps3,
                            lhsT=w3_sb[:, ft, k, :],
                            rhs=xg_sb[:, c, k, :],
                            start=(k == 0), stop=(k == KD - 1),
                        )
                    s_sb = hsb.tile([P, clen], F32, tag="s")
                    nc.scalar.activation(s_sb, ps1,
                                         mybir.ActivationFunctionType.Silu)
                    nc.vector.tensor_mul(h_sb[:, ft, :], s_sb, ps3)

                # ---- down projection: y[d, tok] = w2.T h  (gate applied)
                y_sb = ysb.tile([P, KD, clen], F32, tag="y")
                for dt in range(KD):
                    dsl = slice(dt * P, (dt + 1) * P)
                    psy = ps.tile([P, clen], F32, tag="psy")
                    for ft in range(KF):
                        h2, i = divmod(ft, KH)
                        nc.tensor.matmul(
                            psy,
                            lhsT=w2_sb[:, h2, i, dsl],
                            rhs=h_sb[:, ft, :],
                            start=(ft == 0), stop=(ft == KF - 1),
                        )
                    nc.vector.tensor_mul(y_sb[:, dt, :], psy,
                                         gate_sb[:, csl])
                    if c == N_CHUNKS - 1 and dt % 2 == 1:
                        # last chunk: stream the output out per d-tile pair
                        # so only a sliver of DMA remains after the last MM
                        nc.gpsimd.dma_start(
                            y[c].rearrange("p (k j) -> p k j", k=KD)
                            [:, dt - 1:dt + 1, :],
                            y_sb[:, dt - 1:dt + 1, :])
                if c < N_CHUNKS - 1:
                    nc.gpsimd.dma_start(
                        y[c].rearrange("p (k j) -> p k j", k=KD), y_sb)
    nc.compile()
    return nc


USE_RAW_FFN = True


def _build_ffn_raw_nc(c_pad):
    """Raw-Block (no Tile) variant of the FFN kernel: identical dataflow
    and DRAM layouts as _build_ffn_nc, with hand-placed semaphores.
    Avoids the Tile exit-barrier (~9us) and scheduling slack.

    PSUM banks: ps1/ps3/psy each double-buffered by global group parity;
    every consumer increments a counting semaphore that the producer
    checks before reusing the bank.
    """
    nc = bacc.Bacc("TRN2", target_bir_lowering=False, debug=False,
                   num_devices=N_CORES, enable_partition_id=False)
    clen = c_pad // N_CHUNKS
    assert clen * N_CHUNKS == c_pad and clen <= 512

    xg = nc.dram_tensor("xg", [N_CHUNKS, P, KD * clen], BF16,
                        kind="ExternalInput").ap()
    w1 = nc.dram_tensor("w1", [P, KF * KD * P], BF16,
                        kind="ExternalInput").ap()
    w3 = nc.dram_tensor("w3", [P, KF * KD * P], BF16,
                        kind="ExternalInput").ap()
    w2 = nc.dram_tensor("w2", [NH, P, (KF // NH) * D], BF16,
                        kind="ExternalInput").ap()
    gate = nc.dram_tensor("gate", [1, c_pad], F32, kind="ExternalInput").ap()
    y = nc.dram_tensor("y", [N_CHUNKS, P, KD * clen], F32,
                       kind="ExternalOutput").ap()

    KH = KF // NH
    NPC = len(W_PIECES)
    piece_start = []
    s0 = 0
    for npc in W_PIECES:
        piece_start.append(s0)
        s0 += npc
    start_to_piece = {s: i for i, s in enumerate(piece_start)}

    xg_sb = nc.alloc_sbuf_tensor("xg_sb", [P, N_CHUNKS, KD, clen], BF16).ap()
    w1_sb = nc.alloc_sbuf_tensor("w1_sb", [P, KF, KD, P], BF16).ap()
    w3_sb = nc.alloc_sbuf_tensor("w3_sb", [P, KF, KD, P], BF16).ap()
    w2_sb = nc.alloc_sbuf_tensor("w2_sb", [P, NH, KH, D], BF16).ap()
    gate_sb = nc.alloc_sbuf_tensor("gate_sb", [P, c_pad], F32).ap()
    h_sb = nc.alloc_sbuf_tensor("h_sb", [P, N_CHUNKS, KF, clen], BF16).ap()
    s_sb = nc.alloc_sbuf_tensor("s_sb", [P, 2, clen], F32).ap()
    y_sb = nc.alloc_sbuf_tensor("y_sb", [P, N_CHUNKS, KD, clen], F32).ap()
    warm_sb = nc.alloc_sbuf_tensor("warm_sb", [P, 512], BF16).ap()

    # PE phase schedule: software-pipeline chunks so the down-projection
    # of chunk c runs while chunk c+1's activations are already in flight
    # and never stalls on the tail of its own silu/mul chain.
    PHASES = [("u", 0), ("u", 1), ("d", 0), ("u", 2), ("d", 1), ("d", 2)]

    ps1 = nc.alloc_psum_tensor("ps1", [P, 2, 512], F32).ap()
    ps3 = nc.alloc_psum_tensor("ps3", [P, 2, 512], F32).ap()
    psy = nc.alloc_psum_tensor("psy", [P, 2, 512], F32).ap()
    ps_w = nc.alloc_psum_tensor("ps_w", [P, 512], F32).ap()

    w1s = [nc.alloc_semaphore(f"w1s{i}") for i in range(NPC)]
    w3s = [nc.alloc_semaphore(f"w3s{i}") for i in range(NPC)]
    w2s = [nc.alloc_semaphore(f"w2s{i}") for i in range(NH)]
    xgs = [nc.alloc_semaphore(f"xgs{k}") for k in range(KD)]
    xgc = [nc.alloc_semaphore(f"xgc{c}") for c in range(1, N_CHUNKS)]
    gts = nc.alloc_semaphore("gts")
    ps1_done = nc.alloc_semaphore("ps1_done")
    ps3_done = nc.alloc_semaphore("ps3_done")
    silu_done = nc.alloc_semaphore("silu_done")
    hmul_done = nc.alloc_semaphore("hmul_done")
    psy_done = nc.alloc_semaphore("psy_done")
    ymul_done = nc.alloc_semaphore("ymul_done")
    ydma = nc.alloc_semaphore("ydma")

    xg_r = [xg[c].rearrange("p (k j) -> p k j", k=KD)
            for c in range(N_CHUNKS)]
    w1_r = w1.rearrange("p (t k j) -> p t k j", t=KF, k=KD)
    w3_r = w3.rearrange("p (t k j) -> p t k j", t=KF, k=KD)
    y_r = [y[c].rearrange("p (k j) -> p k j", k=KD) for c in range(N_CHUNKS)]
    gate_bcast = bass.AP(tensor=gate.tensor, offset=gate.offset,
                         ap=[[0, P], gate.ap[1]])
    N_YDMA = N_CHUNKS * KD // 2

    with nc.Block(no_gpsimd_drain=True) as block:
        @block.sync
        def _(sync):
            for pi, npc in enumerate(W_PIECES):
                fts = slice(piece_start[pi], piece_start[pi] + npc)
                sync.dma_start(w1_sb[:, fts], w1_r[:, fts]).then_inc(
                    w1s[pi], 16)
            sync.dma_start(
                w2_sb[:, 0], w2[0].rearrange("p (i d) -> p i d", i=KH),
            ).then_inc(w2s[0], 16)
            # last chunk's outputs per d-tile on the (now idle) HWDGE ring
            # so only a sliver of DMA trails the last matmul
            cl = N_CHUNKS - 1
            for dt in range(KD):
                sync.wait_ge(ymul_done, cl * KD + dt + 1)
                sync.dma_start(
                    y_r[cl][:, dt, :], y_sb[:, cl, dt, :],
                ).then_inc(ydma, 16)
            sync.wait_ge(ydma, (KD // 2 * (N_CHUNKS - 1) + KD) * 16)

        @block.gpsimd
        def _(gpsimd):
            gpsimd.memset(warm_sb, 0)
            for k in range(KD):
                gpsimd.dma_start(xg_sb[:, 0, k], xg_r[0][:, k]).then_inc(
                    xgs[k], 16)
            gpsimd.dma_start(gate_sb, gate_bcast).then_inc(gts, 16)
            for c in range(1, N_CHUNKS):
                gpsimd.dma_start(xg_sb[:, c], xg_r[c]).then_inc(
                    xgc[c - 1], 16)
            for c in range(N_CHUNKS - 1):
                for dp in range(KD // 2):
                    gpsimd.wait_ge(ymul_done, c * KD + 2 * dp + 2)
                    gpsimd.dma_start(
                        y_r[c][:, 2 * dp:2 * dp + 2, :],
                        y_sb[:, c, 2 * dp:2 * dp + 2, :],
                    ).then_inc(ydma, 16)

        @block.scalar
        def _(scalar):
            for pi, npc in enumerate(W_PIECES):
                fts = slice(piece_start[pi], piece_start[pi] + npc)
                scalar.dma_start(w3_sb[:, fts], w3_r[:, fts]).then_inc(
                    w3s[pi], 16)
            scalar.dma_start(
                w2_sb[:, 1], w2[1].rearrange("p (i d) -> p i d", i=KH),
            ).then_inc(w2s[1], 16)
            for c in range(N_CHUNKS):
                for ft in range(KF):
                    idx = c * KF + ft
                    scalar.wait_ge(ps1_done, idx + 1)
                    if idx >= 2:
                        # s_sb slot reused two groups later
                        scalar.wait_ge(hmul_done, idx - 1)
                    scalar.activation(
                        s_sb[:, idx % 2, :clen], ps1[:, idx % 2, :clen],
                        mybir.ActivationFunctionType.Silu,
                    ).then_inc(silu_done, 1)

        @block.vector
        def _(vector):
            # program order mirrors the PE phase schedule so neither
            # engine blocks the other through program-order head-of-line
            for (ph, c) in PHASES:
                if ph == "u":
                    for ft in range(KF):
                        idx = c * KF + ft
                        vector.wait_ge(silu_done, idx + 1)
                        vector.wait_ge(ps3_done, idx + 1)
                        vector.tensor_mul(
                            h_sb[:, c, ft, :clen], s_sb[:, idx % 2, :clen],
                            ps3[:, idx % 2, :clen],
                        ).then_inc(hmul_done, 1)
                else:
                    for dt in range(KD):
                        idx2 = c * KD + dt
                        if idx2 == 0:
                            vector.wait_ge(gts, 16)
                        vector.wait_ge(psy_done, idx2 + 1)
                        vector.tensor_mul(
                            y_sb[:, c, dt, :clen], psy[:, idx2 % 2, :clen],
                            gate_sb[:, c * clen:(c + 1) * clen],
                        ).then_inc(ymul_done, 1)

        @block.tensor
        def _(tensor):
            for _i in range(14):
                tensor.matmul(ps_w[:, :clen], lhsT=warm_sb[:, 0:P],
                              rhs=warm_sb[:, :clen], start=True, stop=True)
            for (ph, c) in PHASES:
                if ph == "u":
                    if c >= 1:
                        tensor.wait_ge(xgc[c - 1], 16)
                    for ft in range(KF):
                        idx = c * KF + ft
                        if c == 0 and ft in start_to_piece:
                            tensor.wait_ge(w1s[start_to_piece[ft]], 16)
                            tensor.wait_ge(w3s[start_to_piece[ft]], 16)
                        if idx >= 2:
                            tensor.wait_ge(silu_done, idx - 1)
                        for k in range(KD):
                            if c == 0 and ft == 0:
                                tensor.wait_ge(xgs[k], 16)
                            mm = tensor.matmul(
                                ps1[:, idx % 2, :clen],
                                lhsT=w1_sb[:, ft, k, :],
                                rhs=xg_sb[:, c, k, :],
                                start=(k == 0), stop=(k == KD - 1),
                            )
                            if k == KD - 1:
                                mm.then_inc(ps1_done, 1)
                        if idx >= 2:
                            tensor.wait_ge(hmul_done, idx - 1)
                        for k in range(KD):
                            mm = tensor.matmul(
                                ps3[:, idx % 2, :clen],
                                lhsT=w3_sb[:, ft, k, :],
                                rhs=xg_sb[:, c, k, :],
                                start=(k == 0), stop=(k == KD - 1),
                            )
                            if k == KD - 1:
                                mm.then_inc(ps3_done, 1)
                else:
                    tensor.wait_ge(hmul_done, (c + 1) * KF)
                    if c == 0:
                        tensor.wait_ge(w2s[0], 16)
                        tensor.wait_ge(w2s[1], 16)
                    for dt in range(KD):
                        idx2 = c * KD + dt
                        if idx2 >= 2:
                            tensor.wait_ge(ymul_done, idx2 - 1)
                        for ft in range(KF):
                            h2, i2 = divmod(ft, KH)
                            mm = tensor.matmul(
                                psy[:, idx2 % 2, :clen],
                                lhsT=w2_sb[:, h2, i2, dt * P:(dt + 1) * P],
                                rhs=h_sb[:, c, ft, :clen],
                                start=(ft == 0), stop=(ft == KF - 1),
                            )
                            if ft == KF - 1:
                                mm.then_inc(psy_done, 1)

    nc.compile()
    return nc


def kernel(x, router_w, w1, w2, w3):
    x = np.asarray(x, dtype=np.float32)
    router_w = np.asarray(router_w, dtype=np.float32)
    w1 = np.asarray(w1, dtype=np.float32)
    w2 = np.asarray(w2, dtype=np.float32)
    w3 = np.asarray(w3, dtype=np.float32)

    x_flat = x.reshape(-1, D)
    core_ids = list(range(N_CORES))

    # ---------------- Launch 1: router logits on-device ----------------
    if "nc" not in _router_cache:
        _router_cache["nc"] = _build_router_nc()
    nc_r = _router_cache["nc"]

    rw_pm = _pm(router_w)  # [P, KD*E]
    in_maps = []
    for c in range(N_CORES):
        shard = x_flat[c * SHARD:(c + 1) * SHARD]
        in_maps.append({
            "xT": _pm(np.ascontiguousarray(shard.T)),
            "rw": rw_pm,
        })
    res_r = run_bass_kernel_spmd(nc_r, in_maps, core_ids)
    logits = np.concatenate(
        [res_r.results[c]["logitsT"].T for c in range(N_CORES)], axis=0)

    # ---------------- Host: routing decisions + loss epilogue ----------------
    lmax = logits.max(axis=-1, keepdims=True)
    ex = np.exp(logits - lmax)
    probs = ex / ex.sum(axis=-1, keepdims=True)

    top1 = np.argmax(probs, axis=-1)
    pm_ = probs.copy()
    pm_[np.arange(N_TOK), top1] = -1.0
    top2 = np.argmax(pm_, axis=-1)
    wa = probs[np.arange(N_TOK), top1]
    wb = probs[np.arange(N_TOK), top2]
    den = wa + wb
    g1 = (wa / den).astype(np.float32)
    g2 = (wb / den).astype(np.float32)

    importance = probs.astype(np.float64).mean(axis=0)
    load = np.bincount(top1, minlength=E).astype(np.float64) / N_TOK
    aux_loss = np.float32(E * np.sum(importance * load) * AUX_COEF)
    z_loss = np.float32(np.mean(logits.astype(np.float64) ** 2) * Z_COEF)

    idx_lists, gate_lists = [], []
    for e in range(E):
        sel = np.where((top1 == e) | (top2 == e))[0]
        gates = np.where(top1[sel] == e, g1[sel], g2[sel]).astype(np.float32)
        idx_lists.append(sel)
        gate_lists.append(gates)

    c_max = max(len(s) for s in idx_lists)
    step = 4 * N_CHUNKS
    c_pad = max(384, -(-c_max // step) * step)
    clen = c_pad // N_CHUNKS

    # ---------------- Launch 2: expert-parallel FFN ----------------
    if c_pad not in _ffn_cache:
        build = _build_ffn_raw_nc if USE_RAW_FFN else _build_ffn_nc
        _ffn_cache[c_pad] = build(c_pad)
    nc_f = _ffn_cache[c_pad]

    bf = ml_dtypes.bfloat16
    KH = KF // NH

    in_maps = []
    for e in range(E):
        sel = idx_lists[e]
        # xg[c][p][k*clen+j] = x[sel[c*clen+j], k*128+p]
        xg_full = np.zeros((D, c_pad), dtype=bf)
        xg_full[:, :len(sel)] = x_flat[sel].T.astype(bf)
        xg_t = (xg_full.reshape(KD, P, N_CHUNKS, clen)
                .transpose(2, 1, 0, 3).reshape(N_CHUNKS, P, KD * clen))
        # w1/w3: [p][ft][k][j] f-tile-major
        w1_t = (w1[e].astype(bf).reshape(KD, P, KF, P)
                .transpose(1, 2, 0, 3).reshape(P, KF * KD * P))
        w3_t = (w3[e].astype(bf).reshape(KD, P, KF, P)
                .transpose(1, 2, 0, 3).reshape(P, KF * KD * P))
        w2_t = (w2[e].astype(bf).reshape(NH, KH, P, D)
                .transpose(0, 2, 1, 3).reshape(NH, P, KH * D))
        gt = np.zeros((1, c_pad), dtype=np.float32)
        gt[0, :len(sel)] = gate_lists[e]
        in_maps.append({
            "xg": np.ascontiguousarray(xg_t),
            "w1": np.ascontiguousarray(w1_t),
            "w3": np.ascontiguousarray(w3_t),
            "w2": np.ascontiguousarray(w2_t),
            "gate": gt,
        })
    res_f = run_bass_kernel_spmd(nc_f, in_maps, core_ids)

    global LAST_EXEC_NS
    LAST_RESULTS["router"] = res_r
    LAST_RESULTS["ffn"] = res_f
    if res_r.exec_time_ns is not None or res_f.exec_time_ns is not None:
        LAST_EXEC_NS = (res_r.exec_time_ns or 0) + (res_f.exec_time_ns or 0)

    # ---------------- Host: combine ----------------
    out = np.zeros((N_TOK, D), dtype=np.float32)
    for e in range(E):
        sel = idx_lists[e]
        ye = res_f.results[e]["y"]  # [N_CHUNKS, P, KD*clen]
        # y[c][p][k*clen+j] -> yT [D, c_pad]
        yT = (ye.reshape(N_CHUNKS, P, KD, clen).transpose(2, 1, 0, 3)
              .reshape(D, c_pad))
        out[sel] += yT[:, :len(sel)].T
    return out.reshape(B, S, D), aux_loss, z_loss


# revision 45
# speedup vs baseline: 1.1526x; 1.1526x over previous
"""MoE layer (top-2 of 8 experts, SwiGLU FFN) on 8 Trainium2 NeuronCores.

Strategy (per spec sharding_hint, expert-parallel):
  Launch 1 (data-parallel router): the 4096 tokens are sharded 512/core;
    each core computes its router logits tile in fp32 on the PE.
  Host dispatch: softmax/top-2/gates + per-expert token index lists are
    derived from the device logits (pure routing decisions + the tiny
    scalar loss epilogue).
  Launch 2 (expert-parallel FFN): core e holds expert e's w1/w3/w2 (bf16)
    and its gathered tokens (bf16); computes
    yT = (silu(x@w1) * (x@w3)) @ w2 * gate fully on-device.
  Host combine: scatter-add the two expert contributions per token.

All heavy math runs on-device; the host only routes/gathers/combines.
All DRAM inputs/outputs use partition-major tiled layouts ([128, ...]
with long contiguous per-partition runs) so every DMA descriptor moves
multi-KB and the transfers run at HBM line rate.
"""

import numpy as np
import ml_dtypes

import concourse.bass as bass
import concourse.tile as tile
from concourse import bacc, mybir
from concourse.bass_utils import run_bass_kernel_spmd

# Problem shapes (hardcoded per contract)
B, S, D, F, E = 2, 2048, 768, 2048, 8
N_TOK = B * S            # 4096
TOP_K = 2
AUX_COEF = 0.01
Z_COEF = 0.001
N_CORES = 8
P = 128                  # SBUF partitions
KD = D // P              # 6  k-tiles over D
KF = F // P              # 16 k-tiles over F
SHARD = N_TOK // N_CORES # 512 tokens/core in the router launch
N_CHUNKS = 3             # equal token chunks per expert in the FFN launch
# w1/w3 stream in f-tile-major pieces: small lead pieces let the PE start
# early, fat pieces amortize DMA fixed cost
W_PIECES = [1, 1, 1, 1, 4, 4, 4]
NH = 2                   # halves for w2 streaming

BF16 = mybir.dt.bfloat16
F32 = mybir.dt.float32

_router_cache = {}
_ffn_cache = {}

# Populated on every kernel() call; test harnesses may read these to report
# HW exec time when NTFF tracing is enabled (BASS_TRACE=1).
LAST_RESULTS = {}
LAST_EXEC_NS = None


def _pm(a, p=P):
    """[R, C] -> partition-major tiled [p, (R//p) * C], row r = t*p + q."""
    r, c = a.shape
    return np.ascontiguousarray(
        a.reshape(r // p, p, c).transpose(1, 0, 2).reshape(p, -1))


def _build_router_nc():
    """Data-parallel router: logitsT[E, tok] = router_w.T @ xT in fp32.

    Raw-block kernel (no Tile) to avoid the Tile exit-barrier cost.
    Transposed formulation: stationary = router_w tile [128, 8], moving =
    xT [128, 512] -> only KD=6 fat matmuls instead of 48 thin ones.
    """
    nc = bacc.Bacc("TRN2", target_bir_lowering=False, debug=False,
                   num_devices=N_CORES, enable_partition_id=False)
    # partition-major: xT_pm[p, k*SHARD + n] = x_shard[n, k*128+p]
    xT = nc.dram_tensor("xT", [P, KD * SHARD], F32, kind="ExternalInput").ap()
    rw = nc.dram_tensor("rw", [P, KD * E], F32, kind="ExternalInput").ap()
    logitsT = nc.dram_tensor("logitsT", [E, SHARD], F32,
                             kind="ExternalOutput").ap()

    xT_sb = nc.alloc_sbuf_tensor("xT_sb", [P, KD, SHARD], F32).ap()
    rw_sb = nc.alloc_sbuf_tensor("rw_sb", [P, KD, E], F32).ap()
    lg_sb = nc.alloc_sbuf_tensor("lg_sb", [E, SHARD], F32).ap()
    ps = nc.alloc_psum_tensor("ps_l", [E, SHARD], F32).ap()

    xT_r = xT.rearrange("p (t n) -> p t n", t=KD)
    rw_r = rw.rearrange("p (t e) -> p t e", t=KD)

    # DMAs on different HWDGE queues complete out of order, so each input
    # DMA gets its own semaphore (a shared counter would race).
    in_sems = [nc.alloc_semaphore(f"in_sem_{k}") for k in range(KD + 1)]

    ps_w = nc.alloc_psum_tensor("ps_warm", [P, P], F32).ap()

    with (
        nc.Block(no_gpsimd_drain=True) as block,
        nc.semaphore("dma_sem") as dma_sem,
        nc.semaphore("mm_sem") as mm_sem,
        nc.semaphore("cp_sem") as cp_sem,
    ):
        @block.sync
        def _(sync):
            sync.dma_start(rw_sb, rw_r).then_inc(in_sems[KD], 16)
            for k in range(0, KD, 2):
                sync.dma_start(xT_sb[:, k, :], xT_r[:, k, :]).then_inc(
                    in_sems[k], 16)
            sync.wait_ge(cp_sem, 1)
            sync.dma_start(logitsT, lg_sb).then_inc(dma_sem, 16)
            sync.wait_ge(dma_sem, 16)

        @block.scalar
        def _(scalar):
            # second HWDGE ring: odd k tiles in parallel with sync's evens
            for k in range(1, KD, 2):
                scalar.dma_start(xT_sb[:, k, :], xT_r[:, k, :]).then_inc(
                    in_sems[k], 16)

        @block.vector
        def _(vector):
            vector.wait_ge(mm_sem, 1)
            vector.tensor_copy(lg_sb, ps).then_inc(cp_sem, 1)

        @block.tensor
        def _(tensor):
            # warm the HAM clock-gate while the input DMAs stream: ~5us
            # of dummy PE activity (values are garbage, results unread)
            for _i in range(12):
                tensor.matmul(ps_w, lhsT=xT_sb[:, 0, 0:P],
                              rhs=xT_sb[:, 1, 0:P], start=True, stop=True)
            tensor.wait_ge(in_sems[KD], 16)
            for k in range(KD):
                tensor.wait_ge(in_sems[k], 16)
                mm = tensor.matmul(
                    ps,
                    lhsT=rw_sb[:, k, :],
                    rhs=xT_sb[:, k, :],
                    start=(k == 0),
                    stop=(k == KD - 1),
                )
                if k == KD - 1:
                    mm.then_inc(mm_sem, 1)

    nc.compile()
    return nc


def _build_ffn_nc(c_pad):
    """Expert-parallel SwiGLU FFN over gathered tokens.

    Partition-major DRAM layouts (ft = 128-wide f tile, h = f-half,
    c = chunk):
      xg   [N_CHUNKS, P, KD*clen]   bf16   xg[c][p][k*clen+j]  = x[tok, d]
      w1/3 [P, KF*KD*128]           bf16   [p][ft][k][j] = w[k*128+p, ft*128+j]
      w2   [NH, P, 8*768]           bf16   w2[h][p][i*768+d]   = w2[(h*8+i)*128+p, d]
      gate [1, c_pad]               f32    (partition-broadcast on load)
      y    [N_CHUNKS, P, KD*clen]   f32    y[c][p][k*clen+j]   = out[tok, d]
    """
    nc = bacc.Bacc("TRN2", target_bir_lowering=False, debug=False,
                   num_devices=N_CORES, enable_partition_id=False)
    clen = c_pad // N_CHUNKS
    assert clen * N_CHUNKS == c_pad and clen <= 512

    xg = nc.dram_tensor("xg", [N_CHUNKS, P, KD * clen], BF16,
                        kind="ExternalInput").ap()
    w1 = nc.dram_tensor("w1", [P, KF * KD * P], BF16,
                        kind="ExternalInput").ap()
    w3 = nc.dram_tensor("w3", [P, KF * KD * P], BF16,
                        kind="ExternalInput").ap()
    w2 = nc.dram_tensor("w2", [NH, P, (KF // NH) * D], BF16,
                        kind="ExternalInput").ap()
    gate = nc.dram_tensor("gate", [1, c_pad], F32, kind="ExternalInput").ap()
    y = nc.dram_tensor("y", [N_CHUNKS, P, KD * clen], F32,
                       kind="ExternalOutput").ap()

    KH = KF // NH         # 8

    with tile.TileContext(nc) as tc:
        with (
            tc.tile_pool(name="wsb", bufs=1) as wsb,
            tc.tile_pool(name="hsb", bufs=2) as hsb,
            tc.tile_pool(name="ysb", bufs=2) as ysb,
            tc.tile_pool(name="ps", bufs=2, space="PSUM") as ps,
        ):
            # Resident SBUF tensors.  Inputs stream over BOTH HWDGE rings
            # (sync + scalar) in PE consume-order; gate/output use the
            # SWDGE (gpsimd) path so they don't contend with the rings.
            xg_sb = wsb.tile([P, N_CHUNKS, KD, clen], BF16, tag="xg")
            w1_sb = wsb.tile([P, KF, KD, P], BF16, tag="w1")
            w3_sb = wsb.tile([P, KF, KD, P], BF16, tag="w3")
            w2_sb = wsb.tile([P, NH, KH, D], BF16, tag="w2")
            gate_sb = wsb.tile([P, c_pad], F32, tag="gate")

            # PE pre-warm: dummy matmuls on a zeroed tile keep the HAM
            # clock-gate busy while the first input DMAs stream in.
            warm_sb = hsb.tile([P, 512], BF16, tag="warm")
            nc.gpsimd.memset(warm_sb, 0)
            for _i in range(14):
                ps_w = ps.tile([P, 512], F32, tag="pswarm")
                nc.tensor.matmul(ps_w, lhsT=warm_sb[:, 0:P],
                                 rhs=warm_sb, start=True, stop=True)

            # three parallel DMA paths: w1 on the sync HWDGE ring, w3 on
            # the scalar HWDGE ring, activations/gate/output on SWDGE.
            # chunk-0 activations stream per k so the first matmul only
            # waits for one 90KB slice.
            xg0 = xg[0].rearrange("p (k j) -> p k j", k=KD)
            for k in range(KD):
                nc.gpsimd.dma_start(xg_sb[:, 0, k], xg0[:, k])
            w1_r = w1.rearrange("p (t k j) -> p t k j", t=KF, k=KD)
            w3_r = w3.rearrange("p (t k j) -> p t k j", t=KF, k=KD)
            ft0 = 0
            for npc in W_PIECES:
                fts = slice(ft0, ft0 + npc)
                nc.sync.dma_start(w1_sb[:, fts], w1_r[:, fts])
                nc.scalar.dma_start(w3_sb[:, fts], w3_r[:, fts])
                ft0 += npc
            for c in range(1, N_CHUNKS):
                nc.gpsimd.dma_start(
                    xg_sb[:, c], xg[c].rearrange("p (k j) -> p k j", k=KD))
            nc.sync.dma_start(
                w2_sb[:, 0], w2[0].rearrange("p (i d) -> p i d", i=KH))
            nc.scalar.dma_start(
                w2_sb[:, 1], w2[1].rearrange("p (i d) -> p i d", i=KH))

            gate_bcast = bass.AP(
                tensor=gate.tensor, offset=gate.offset,
                ap=[[0, P], gate.ap[1]],
            )
            nc.gpsimd.dma_start(gate_sb, gate_bcast)

            for c in range(N_CHUNKS):
                csl = slice(c * clen, (c + 1) * clen)
                # ---- up projections: hT[f, tok] = silu(w1.T x) * (w3.T x)
                h_sb = hsb.tile([P, KF, clen], BF16, tag="h")
                for ft in range(KF):
                    ps1 = ps.tile([P, clen], F32, tag="ps1")
                    for k in range(KD):
                        nc.tensor.matmul(
                            ps1,
                            lhsT=w1_sb[:, ft, k, :],
                            rhs=xg_sb[:, c, k, :],
                            start=(k == 0), stop=(k == KD - 1),
                        )
                    ps3 = ps.tile([P, clen], F32, tag="ps3")
                    for k in range(KD):
                        nc.tensor.matmul(
                            ps3,
                            lhsT=w3_sb[:, ft, k, :],
                            rhs=xg_sb[:, c, k, :],
                            start=(k == 0), stop=(k == KD - 1),
                        )
                    s_sb = hsb.tile([P, clen], F32, tag="s")
                    nc.scalar.activation(s_sb, ps1,
                                         mybir.ActivationFunctionType.Silu)
                    nc.vector.tensor_mul(h_sb[:, ft, :], s_sb, ps3)

                # ---- down projection: y[d, tok] = w2.T h  (gate applied)
                y_sb = ysb.tile([P, KD, clen], F32, tag="y")
                for dt in range(KD):
                    dsl = slice(dt * P, (dt + 1) * P)
                    psy = ps.tile([P, clen], F32, tag="psy")
                    for ft in range(KF):
                        h2, i = divmod(ft, KH)
                        nc.tensor.matmul(
                            psy,
                            lhsT=w2_sb[:, h2, i, dsl],
                            rhs=h_sb[:, ft, :],
                            start=(ft == 0), stop=(ft == KF - 1),
                        )
                    nc.vector.tensor_mul(y_sb[:, dt, :], psy,
                                         gate_sb[:, csl])
                    if c == N_CHUNKS - 1 and dt % 2 == 1:
                        # last chunk: stream the output out per d-tile pair
                        # so only a sliver of DMA remains after the last MM
                        nc.gpsimd.dma_start(
                            y[c].rearrange("p (k j) -> p k j", k=KD)
                            [:, dt - 1:dt + 1, :],
                            y_sb[:, dt - 1:dt + 1, :])
                if c < N_CHUNKS - 1:
                    nc.gpsimd.dma_start(
                        y[c].rearrange("p (k j) -> p k j", k=KD), y_sb)
    nc.compile()
    return nc


USE_RAW_FFN = True


def _build_ffn_raw_nc(c_pad):
    """Raw-Block (no Tile) variant of the FFN kernel: identical dataflow
    and DRAM layouts as _build_ffn_nc, with hand-placed semaphores.
    Avoids the Tile exit-barrier (~9us) and scheduling slack.

    PSUM banks: ps1/ps3/psy each double-buffered by global group parity;
    every consumer increments a counting semaphore that the producer
    checks before reusing the bank.
    """
    nc = bacc.Bacc("TRN2", target_bir_lowering=False, debug=False,
                   num_devices=N_CORES, enable_partition_id=False)
    clen = c_pad // N_CHUNKS
    assert clen * N_CHUNKS == c_pad and clen <= 512

    xg = nc.dram_tensor("xg", [N_CHUNKS, P, KD * clen], BF16,
                        kind="ExternalInput").ap()
    w1 = nc.dram_tensor("w1", [P, KF * KD * P], BF16,
                        kind="ExternalInput").ap()
    w3 = nc.dram_tensor("w3", [P, KF * KD * P], BF16,
                        kind="ExternalInput").ap()
    w2 = nc.dram_tensor("w2", [NH, P, (KF // NH) * D], BF16,
                        kind="ExternalInput").ap()
    gate = nc.dram_tensor("gate", [1, c_pad], F32, kind="ExternalInput").ap()
    y = nc.dram_tensor("y", [N_CHUNKS, P, KD * clen], F32,
                       kind="ExternalOutput").ap()

    KH = KF // NH
    NPC = len(W_PIECES)
    piece_start = []
    s0 = 0
    for npc in W_PIECES:
        piece_start.append(s0)
        s0 += npc
    start_to_piece = {s: i for i, s in enumerate(piece_start)}

    xg_sb = nc.alloc_sbuf_tensor("xg_sb", [P, N_CHUNKS, KD, clen], BF16).ap()
    w1_sb = nc.alloc_sbuf_tensor("w1_sb", [P, KF, KD, P], BF16).ap()
    w3_sb = nc.alloc_sbuf_tensor("w3_sb", [P, KF, KD, P], BF16).ap()
    w2_sb = nc.alloc_sbuf_tensor("w2_sb", [P, NH, KH, D], BF16).ap()
    gate_sb = nc.alloc_sbuf_tensor("gate_sb", [P, c_pad], F32).ap()
    h_sb = nc.alloc_sbuf_tensor("h_sb", [P, N_CHUNKS, KF, clen], BF16).ap()
    s_sb = nc.alloc_sbuf_tensor("s_sb", [P, 2, clen], F32).ap()
    y_sb = nc.alloc_sbuf_tensor("y_sb", [P, N_CHUNKS, KD, clen], F32).ap()
    warm_sb = nc.alloc_sbuf_tensor("warm_sb", [P, 512], BF16).ap()

    # PE phase schedule: software-pipeline chunks so the down-projection
    # of chunk c runs while chunk c+1's activations are already in flight
    # and never stalls on the tail of its own silu/mul chain.
    PHASES = [("u", 0), ("u", 1), ("d", 0), ("u", 2), ("d", 1), ("d", 2)]

    ps1 = nc.alloc_psum_tensor("ps1", [P, 2, 512], F32).ap()
    ps3 = nc.alloc_psum_tensor("ps3", [P, 2, 512], F32).ap()
    psy = nc.alloc_psum_tensor("psy", [P, 2, 512], F32).ap()
    ps_w = nc.alloc_psum_tensor("ps_w", [P, 512], F32).ap()

    w1s = [nc.alloc_semaphore(f"w1s{i}") for i in range(NPC)]
    w3s = [nc.alloc_semaphore(f"w3s{i}") for i in range(NPC)]
    w2s = [nc.alloc_semaphore(f"w2s{i}") for i in range(NH)]
    xgs = [nc.alloc_semaphore(f"xgs{k}") for k in range(KD)]
    xgc = [nc.alloc_semaphore(f"xgc{c}") for c in range(1, N_CHUNKS)]
    gts = nc.alloc_semaphore("gts")
    ps1_done = nc.alloc_semaphore("ps1_done")
    ps3_done = nc.alloc_semaphore("ps3_done")
    silu_done = nc.alloc_semaphore("silu_done")
    hmul_done = nc.alloc_semaphore("hmul_done")
    psy_done = nc.alloc_semaphore("psy_done")
    ymul_done = nc.alloc_semaphore("ymul_done")
    ydma = nc.alloc_semaphore("ydma")

    xg_r = [xg[c].rearrange("p (k j) -> p k j", k=KD)
            for c in range(N_CHUNKS)]
    w1_r = w1.rearrange("p (t k j) -> p t k j", t=KF, k=KD)
    w3_r = w3.rearrange("p (t k j) -> p t k j", t=KF, k=KD)
    y_r = [y[c].rearrange("p (k j) -> p k j", k=KD) for c in range(N_CHUNKS)]
    gate_bcast = bass.AP(tensor=gate.tensor, offset=gate.offset,
                         ap=[[0, P], gate.ap[1]])
    N_YDMA = N_CHUNKS * KD // 2

    with nc.Block(no_gpsimd_drain=True) as block:
        @block.sync
        def _(sync):
            for pi, npc in enumerate(W_PIECES):
                fts = slice(piece_start[pi], piece_start[pi] + npc)
                sync.dma_start(w1_sb[:, fts], w1_r[:, fts]).then_inc(
                    w1s[pi], 16)
            sync.dma_start(
                w2_sb[:, 0], w2[0].rearrange("p (i d) -> p i d", i=KH),
            ).then_inc(w2s[0], 16)
            # last chunk's outputs per d-tile on the (now idle) HWDGE ring
            # so only a sliver of DMA trails the last matmul
            cl = N_CHUNKS - 1
            for dt in range(KD):
                sync.wait_ge(ymul_done, cl * KD + dt + 1)
                sync.dma_start(
                    y_r[cl][:, dt, :], y_sb[:, cl, dt, :],
                ).then_inc(ydma, 16)
            sync.wait_ge(ydma, (KD // 2 * (N_CHUNKS - 1) + KD) * 16)

        @block.gpsimd
        def _(gpsimd):
            gpsimd.memset(warm_sb, 0)
            for k in range(KD):
                gpsimd.dma_start(xg_sb[:, 0, k], xg_r[0][:, k]).then_inc(
                    xgs[k], 16)
            gpsimd.dma_start(gate_sb, gate_bcast).then_inc(gts, 16)
            for c in range(1, N_CHUNKS):
                gpsimd.dma_start(xg_sb[:, c], xg_r[c]).then_inc(
                    xgc[c - 1], 16)
            for c in range(N_CHUNKS - 1):
                for dp in range(KD // 2):
                    gpsimd.wait_ge(ymul_done, c * KD + 2 * dp + 2)
                    gpsimd.dma_start(
                        y_r[c][:, 2 * dp:2 * dp + 2, :],
                        y_sb[:, c, 2 * dp:2 * dp + 2, :],
                    ).then_inc(ydma, 16)

        @block.scalar
        def _(scalar):
            for pi, npc in enumerate(W_PIECES):
                fts = slice(piece_start[pi], piece_start[pi] + npc)
                scalar.dma_start(w3_sb[:, fts], w3_r[:, fts]).then_inc(
                    w3s[pi], 16)
            scalar.dma_start(
                w2_sb[:, 1], w2[1].rearrange("p (i d) -> p i d", i=KH),
            ).then_inc(w2s[1], 16)
            for c in range(N_CHUNKS):
                for ft in range(KF):
                    idx = c * KF + ft
                    scalar.wait_ge(ps1_done, idx + 1)
                    if idx >= 2:
                        # s_sb slot reused two groups later
                        scalar.wait_ge(hmul_done, idx - 1)
                    scalar.activation(
                        s_sb[:, idx % 2, :clen], ps1[:, idx % 2, :clen],
                        mybir.ActivationFunctionType.Silu,
                    ).then_inc(silu_done, 1)

        @block.vector
        def _(vector):
            # program order mirrors the PE phase schedule so neither
            # engine blocks the other through program-order head-of-line
            for (ph, c) in PHASES:
                if ph == "u":
                    for ft in range(KF):
                        idx = c * KF + ft
                        vector.wait_ge(silu_done, idx + 1)
                        vector.wait_ge(ps3_done, idx + 1)
                        vector.tensor_mul(
                            h_sb[:, c, ft, :clen], s_sb[:, idx % 2, :clen],
                            ps3[:, idx % 2, :clen],
                        ).then_inc(hmul_done, 1)
                else:
                    for dt in range(KD):
                        idx2 = c * KD + dt
                        if idx2 == 0:
                            vector.wait_ge(gts, 16)
                        vector.wait_ge(psy_done, idx2 + 1)
                        vector.tensor_mul(
                            y_sb[:, c, dt, :clen], psy[:, idx2 % 2, :clen],
                            gate_sb[:, c * clen:(c + 1) * clen],
                        ).then_inc(ymul_done, 1)

        @block.tensor
        def _(tensor):
            for _i in range(14):
                tensor.matmul(ps_w[:, :clen], lhsT=warm_sb[:, 0:P],
                              rhs=warm_sb[:, :clen], start=True, stop=True)
            for (ph, c) in PHASES:
                if ph == "u":
                    if c >= 1:
                        tensor.wait_ge(xgc[c - 1], 16)
                    for ft in range(KF):
                        idx = c * KF + ft
                        if c == 0 and ft in start_to_piece:
                            tensor.wait_ge(w1s[start_to_piece[ft]], 16)
                            tensor.wait_ge(w3s[start_to_piece[ft]], 16)
                        if idx >= 2:
                            tensor.wait_ge(silu_done, idx - 1)
                        for k in range(KD):
                            if c == 0 and ft == 0:
                                tensor.wait_ge(xgs[k], 16)
                            mm = tensor.matmul(
                                ps1[:, idx % 2, :clen],
                                lhsT=w1_sb[:, ft, k, :],
                                rhs=xg_sb[:, c, k, :],
                                start=(k == 0), stop=(k == KD - 1),
                            )
                            if k == KD - 1:
                                mm.then_inc(ps1_done, 1)
                        if idx >= 2:
                            tensor.wait_ge(hmul_done, idx - 1)
                        for k in range(KD):
                            mm = tensor.matmul(
                                ps3[:, idx % 2, :clen],
                                lhsT=w3_sb[:, ft, k, :],
                                rhs=xg_sb[:, c, k, :],
                                start=(k == 0), stop=(k == KD - 1),
                            )
                            if k == KD - 1:
                                mm.then_inc(ps3_done, 1)
                else:
                    tensor.wait_ge(hmul_done, (c + 1) * KF)
                    if c == 0:
                        tensor.wait_ge(w2s[0], 16)
                        tensor.wait_ge(w2s[1], 16)
                    for dt in range(KD):
                        idx2 = c * KD + dt
                        if idx2 >= 2:
                            tensor.wait_ge(ymul_done, idx2 - 1)
                        for ft in range(KF):
                            h2, i2 = divmod(ft, KH)
                            mm = tensor.matmul(
                                psy[:, idx2 % 2, :clen],
                                lhsT=w2_sb[:, h2, i2, dt * P:(dt + 1) * P],
                                rhs=h_sb[:, c, ft, :clen],
                                start=(ft == 0), stop=(ft == KF - 1),
                            )
                            if ft == KF - 1:
                                mm.then_inc(psy_done, 1)

    nc.compile()
    return nc


def kernel(x, router_w, w1, w2, w3):
    x = np.asarray(x, dtype=np.float32)
    router_w = np.asarray(router_w, dtype=np.float32)
    w1 = np.asarray(w1, dtype=np.float32)
    w2 = np.asarray(w2, dtype=np.float32)
    w3 = np.asarray(w3, dtype=np.float32)

    x_flat = x.reshape(-1, D)
    core_ids = list(range(N_CORES))

    # ---------------- Launch 1: router logits on-device ----------------
    if "nc" not in _router_cache:
        _router_cache["nc"] = _build_router_nc()
    nc_r = _router_cache["nc"]

    rw_pm = _pm(router_w)  # [P, KD*E]
    in_maps = []
    for c in range(N_CORES):
        shard = x_flat[c * SHARD:(c + 1) * SHARD]
        in_maps.append({
            "xT": _pm(np.ascontiguousarray(shard.T)),
            "rw": rw_pm,
        })
    res_r = run_bass_kernel_spmd(nc_r, in_maps, core_ids)
    logits = np.concatenate(
        [res_r.results[c]["logitsT"].T for c in range(N_CORES)], axis=0)

    # ---------------- Host: routing decisions + loss epilogue ----------------
    lmax = logits.max(axis=-1, keepdims=True)
    ex = np.exp(logits - lmax)
    probs = ex / ex.sum(axis=-1, keepdims=True)

    top1 = np.argmax(probs, axis=-1)
    pm_ = probs.copy()
    pm_[np.arange(N_TOK), top1] = -1.0
    top2 = np.argmax(pm_, axis=-1)
    wa = probs[np.arange(N_TOK), top1]
    wb = probs[np.arange(N_TOK), top2]
    den = wa + wb
    g1 = (wa / den).astype(np.float32)
    g2 = (wb / den).astype(np.float32)

    importance = probs.astype(np.float64).mean(axis=0)
    load = np.bincount(top1, minlength=E).astype(np.float64) / N_TOK
    aux_loss = np.float32(E * np.sum(importance * load) * AUX_COEF)
    z_loss = np.float32(np.mean(logits.astype(np.float64) ** 2) * Z_COEF)

    idx_lists, gate_lists = [], []
    for e in range(E):
        sel = np.where((top1 == e) | (top2 == e))[0]
        gates = np.where(top1[sel] == e, g1[sel], g2[sel]).astype(np.float32)
        idx_lists.append(sel)
        gate_lists.append(gates)

    c_max = max(len(s) for s in idx_lists)
    step = 4 * N_CHUNKS
    c_pad = max(384, -(-c_max // step) * step)
    clen = c_pad // N_CHUNKS

    # ---------------- Launch 2: expert-parallel FFN ----------------
    if c_pad not in _ffn_cache:
        build = _build_ffn_raw_nc if USE_RAW_FFN else _build_ffn_nc
        _ffn_cache[c_pad] = build(c_pad)
    nc_f = _ffn_cache[c_pad]

    bf = ml_dtypes.bfloat16
    KH = KF // NH

    in_maps = []
    for e in range(E):
        sel = idx_lists[e]
        # xg[c][p][k*clen+j] = x[sel[c*clen+j], k*128+p]
        xg_full = np.zeros((D, c_pad), dtype=bf)
        xg_full[:, :len(sel)] = x_flat[sel].T.astype(bf)
        xg_t = (xg_full.reshape(KD, P, N_CHUNKS, clen)
                .transpose(2, 1, 0, 3).reshape(N_CHUNKS, P, KD * clen))
        # w1/w3: [p][ft][k][j] f-tile-major
        w1_t = (w1[e].astype(bf).reshape(KD, P, KF, P)
                .transpose(1, 2, 0, 3).reshape(P, KF * KD * P))
        w3_t = (w3[e].astype(bf).reshape(KD, P, KF, P)
                .transpose(1, 2, 0, 3).reshape(P, KF * KD * P))
        w2_t = (w2[e].astype(bf).reshape(NH, KH, P, D)
                .transpose(0, 2, 1, 3).reshape(NH, P, KH * D))
        gt = np.zeros((1, c_pad), dtype=np.float32)
        gt[0, :len(sel)] = gate_lists[e]
        in_maps.append({
            "xg": np.ascontiguousarray(xg_t),
            "w1": np.ascontiguousarray(w1_t),
            "w3": np.ascontiguousarray(w3_t),
            "w2": np.ascontiguousarray(w2_t),
            "gate": gt,
        })
    res_f = run_bass_kernel_spmd(nc_f, in_maps, core_ids)

    global LAST_EXEC_NS
    LAST_RESULTS["router"] = res_r
    LAST_RESULTS["ffn"] = res_f
    if res_r.exec_time_ns is not None or res_f.exec_time_ns is not None:
        LAST_EXEC_NS = (res_r.exec_time_ns or 0) + (res_f.exec_time_ns or 0)

    # ---------------- Host: combine ----------------
    out = np.zeros((N_TOK, D), dtype=np.float32)
    for e in range(E):
        sel = idx_lists[e]
        ye = res_f.results[e]["y"]  # [N_CHUNKS, P, KD*clen]
        # y[c][p][k*clen+j] -> yT [D, c_pad]
        yT = (ye.reshape(N_CHUNKS, P, KD, clen).transpose(2, 1, 0, 3)
              .reshape(D, c_pad))
        out[sel] += yT[:, :len(sel)].T
    return out.reshape(B, S, D), aux_loss, z_loss


# revision 46
# speedup vs baseline: 1.1715x; 1.0165x over previous
"""MoE layer (top-2 of 8 experts, SwiGLU FFN) on 8 Trainium2 NeuronCores.

Strategy (per spec sharding_hint, expert-parallel):
  Launch 1 (data-parallel router): the 4096 tokens are sharded 512/core;
    each core computes its router logits tile in fp32 on the PE.
  Host dispatch: softmax/top-2/gates + per-expert token index lists are
    derived from the device logits (pure routing decisions + the tiny
    scalar loss epilogue).
  Launch 2 (expert-parallel FFN): core e holds expert e's w1/w3/w2 (bf16)
    and its gathered tokens (bf16); computes
    yT = (silu(x@w1) * (x@w3)) @ w2 * gate fully on-device.
  Host combine: scatter-add the two expert contributions per token.

All heavy math runs on-device; the host only routes/gathers/combines.
All DRAM inputs/outputs use partition-major tiled layouts ([128, ...]
with long contiguous per-partition runs) so every DMA descriptor moves
multi-KB and the transfers run at HBM line rate.
"""

import numpy as np
import ml_dtypes

import concourse.bass as bass
import concourse.tile as tile
from concourse import bacc, mybir
from concourse.bass_utils import run_bass_kernel_spmd

# Problem shapes (hardcoded per contract)
B, S, D, F, E = 2, 2048, 768, 2048, 8
N_TOK = B * S            # 4096
TOP_K = 2
AUX_COEF = 0.01
Z_COEF = 0.001
N_CORES = 8
P = 128                  # SBUF partitions
KD = D // P              # 6  k-tiles over D
KF = F // P              # 16 k-tiles over F
SHARD = N_TOK // N_CORES # 512 tokens/core in the router launch
N_CHUNKS = 3             # equal token chunks per expert in the FFN launch
# w1/w3 stream in f-tile-major pieces: small lead pieces let the PE start
# early, fat pieces amortize DMA fixed cost
W_PIECES = [1, 1, 1, 1, 4, 4, 4]
NH = 2                   # halves for w2 streaming

BF16 = mybir.dt.bfloat16
F32 = mybir.dt.float32

_router_cache = {}
_ffn_cache = {}

# Populated on every kernel() call; test harnesses may read these to report
# HW exec time when NTFF tracing is enabled (BASS_TRACE=1).
LAST_RESULTS = {}
LAST_EXEC_NS = None


def _pm(a, p=P):
    """[R, C] -> partition-major tiled [p, (R//p) * C], row r = t*p + q."""
    r, c = a.shape
    return np.ascontiguousarray(
        a.reshape(r // p, p, c).transpose(1, 0, 2).reshape(p, -1))


def _build_router_nc():
    """Data-parallel router: logitsT[E, tok] = router_w.T @ xT in fp32.

    Raw-block kernel (no Tile) to avoid the Tile exit-barrier cost.
    Transposed formulation: stationary = router_w tile [128, 8], moving =
    xT [128, 512] -> only KD=6 fat matmuls instead of 48 thin ones.
    """
    nc = bacc.Bacc("TRN2", target_bir_lowering=False, debug=False,
                   num_devices=N_CORES, enable_partition_id=False)
    # partition-major: xT_pm[p, k*SHARD + n] = x_shard[n, k*128+p]
    xT = nc.dram_tensor("xT", [P, KD * SHARD], F32, kind="ExternalInput").ap()
    rw = nc.dram_tensor("rw", [P, KD * E], F32, kind="ExternalInput").ap()
    logitsT = nc.dram_tensor("logitsT", [E, SHARD], F32,
                             kind="ExternalOutput").ap()

    xT_sb = nc.alloc_sbuf_tensor("xT_sb", [P, KD, SHARD], F32).ap()
    rw_sb = nc.alloc_sbuf_tensor("rw_sb", [P, KD, E], F32).ap()
    lg_sb = nc.alloc_sbuf_tensor("lg_sb", [E, SHARD], F32).ap()
    ps = nc.alloc_psum_tensor("ps_l", [E, SHARD], F32).ap()

    xT_r = xT.rearrange("p (t n) -> p t n", t=KD)
    rw_r = rw.rearrange("p (t e) -> p t e", t=KD)

    # DMAs on different HWDGE queues complete out of order, so each input
    # DMA gets its own semaphore (a shared counter would race).
    in_sems = [nc.alloc_semaphore(f"in_sem_{k}") for k in range(KD + 1)]

    ps_w = nc.alloc_psum_tensor("ps_warm", [P, P], F32).ap()

    with (
        nc.Block(no_gpsimd_drain=True) as block,
        nc.semaphore("dma_sem") as dma_sem,
        nc.semaphore("mm_sem") as mm_sem,
        nc.semaphore("cp_sem") as cp_sem,
    ):
        @block.sync
        def _(sync):
            sync.dma_start(rw_sb, rw_r).then_inc(in_sems[KD], 16)
            for k in range(0, KD, 2):
                sync.dma_start(xT_sb[:, k, :], xT_r[:, k, :]).then_inc(
                    in_sems[k], 16)
            sync.wait_ge(cp_sem, 1)
            sync.dma_start(logitsT, lg_sb).then_inc(dma_sem, 16)
            sync.wait_ge(dma_sem, 16)

        @block.scalar
        def _(scalar):
            # second HWDGE ring: odd k tiles in parallel with sync's evens
            for k in range(1, KD, 2):
                scalar.dma_start(xT_sb[:, k, :], xT_r[:, k, :]).then_inc(
                    in_sems[k], 16)

        @block.vector
        def _(vector):
            vector.wait_ge(mm_sem, 1)
            vector.tensor_copy(lg_sb, ps).then_inc(cp_sem, 1)

        @block.tensor
        def _(tensor):
            # warm the HAM clock-gate while the input DMAs stream: ~3.8us
            # of dummy PE activity (values are garbage, results unread)
            for _i in range(9):
                tensor.matmul(ps_w, lhsT=xT_sb[:, 0, 0:P],
                              rhs=xT_sb[:, 1, 0:P], start=True, stop=True)
            tensor.wait_ge(in_sems[KD], 16)
            for k in range(KD):
                tensor.wait_ge(in_sems[k], 16)
                mm = tensor.matmul(
                    ps,
                    lhsT=rw_sb[:, k, :],
                    rhs=xT_sb[:, k, :],
                    start=(k == 0),
                    stop=(k == KD - 1),
                )
                if k == KD - 1:
                    mm.then_inc(mm_sem, 1)

    nc.compile()
    return nc


def _build_ffn_nc(c_pad):
    """Expert-parallel SwiGLU FFN over gathered tokens.

    Partition-major DRAM layouts (ft = 128-wide f tile, h = f-half,
    c = chunk):
      xg   [N_CHUNKS, P, KD*clen]   bf16   xg[c][p][k*clen+j]  = x[tok, d]
      w1/3 [P, KF*KD*128]           bf16   [p][ft][k][j] = w[k*128+p, ft*128+j]
      w2   [NH, P, 8*768]           bf16   w2[h][p][i*768+d]   = w2[(h*8+i)*128+p, d]
      gate [1, c_pad]               f32    (partition-broadcast on load)
      y    [N_CHUNKS, P, KD*clen]   f32    y[c][p][k*clen+j]   = out[tok, d]
    """
    nc = bacc.Bacc("TRN2", target_bir_lowering=False, debug=False,
                   num_devices=N_CORES, enable_partition_id=False)
    clen = c_pad // N_CHUNKS
    assert clen * N_CHUNKS == c_pad and clen <= 512

    xg = nc.dram_tensor("xg", [N_CHUNKS, P, KD * clen], BF16,
                        kind="ExternalInput").ap()
    w1 = nc.dram_tensor("w1", [P, KF * KD * P], BF16,
                        kind="ExternalInput").ap()
    w3 = nc.dram_tensor("w3", [P, KF * KD * P], BF16,
                        kind="ExternalInput").ap()
    w2 = nc.dram_tensor("w2", [NH, P, (KF // NH) * D], BF16,
                        kind="ExternalInput").ap()
    gate = nc.dram_tensor("gate", [1, c_pad], F32, kind="ExternalInput").ap()
    y = nc.dram_tensor("y", [N_CHUNKS, P, KD * clen], F32,
                       kind="ExternalOutput").ap()

    KH = KF // NH         # 8

    with tile.TileContext(nc) as tc:
        with (
            tc.tile_pool(name="wsb", bufs=1) as wsb,
            tc.tile_pool(name="hsb", bufs=2) as hsb,
            tc.tile_pool(name="ysb", bufs=2) as ysb,
            tc.tile_pool(name="ps", bufs=2, space="PSUM") as ps,
        ):
            # Resident SBUF tensors.  Inputs stream over BOTH HWDGE rings
            # (sync + scalar) in PE consume-order; gate/output use the
            # SWDGE (gpsimd) path so they don't contend with the rings.
            xg_sb = wsb.tile([P, N_CHUNKS, KD, clen], BF16, tag="xg")
            w1_sb = wsb.tile([P, KF, KD, P], BF16, tag="w1")
            w3_sb = wsb.tile([P, KF, KD, P], BF16, tag="w3")
            w2_sb = wsb.tile([P, NH, KH, D], BF16, tag="w2")
            gate_sb = wsb.tile([P, c_pad], F32, tag="gate")

            # PE pre-warm: dummy matmuls on a zeroed tile keep the HAM
            # clock-gate busy while the first input DMAs stream in.
            warm_sb = hsb.tile([P, 512], BF16, tag="warm")
            nc.gpsimd.memset(warm_sb, 0)
            for _i in range(14):
                ps_w = ps.tile([P, 512], F32, tag="pswarm")
                nc.tensor.matmul(ps_w, lhsT=warm_sb[:, 0:P],
                                 rhs=warm_sb, start=True, stop=True)

            # three parallel DMA paths: w1 on the sync HWDGE ring, w3 on
            # the scalar HWDGE ring, activations/gate/output on SWDGE.
            # chunk-0 activations stream per k so the first matmul only
            # waits for one 90KB slice.
            xg0 = xg[0].rearrange("p (k j) -> p k j", k=KD)
            for k in range(KD):
                nc.gpsimd.dma_start(xg_sb[:, 0, k], xg0[:, k])
            w1_r = w1.rearrange("p (t k j) -> p t k j", t=KF, k=KD)
            w3_r = w3.rearrange("p (t k j) -> p t k j", t=KF, k=KD)
            ft0 = 0
            for npc in W_PIECES:
                fts = slice(ft0, ft0 + npc)
                nc.sync.dma_start(w1_sb[:, fts], w1_r[:, fts])
                nc.scalar.dma_start(w3_sb[:, fts], w3_r[:, fts])
                ft0 += npc
            for c in range(1, N_CHUNKS):
                nc.gpsimd.dma_start(
                    xg_sb[:, c], xg[c].rearrange("p (k j) -> p k j", k=KD))
            nc.sync.dma_start(
                w2_sb[:, 0], w2[0].rearrange("p (i d) -> p i d", i=KH))
            nc.scalar.dma_start(
                w2_sb[:, 1], w2[1].rearrange("p (i d) -> p i d", i=KH))

            gate_bcast = bass.AP(
                tensor=gate.tensor, offset=gate.offset,
                ap=[[0, P], gate.ap[1]],
            )
            nc.gpsimd.dma_start(gate_sb, gate_bcast)

            for c in range(N_CHUNKS):
                csl = slice(c * clen, (c + 1) * clen)
                # ---- up projections: hT[f, tok] = silu(w1.T x) * (w3.T x)
                h_sb = hsb.tile([P, KF, clen], BF16, tag="h")
                for ft in range(KF):
                    ps1 = ps.tile([P, clen], F32, tag="ps1")
                    for k in range(KD):
                        nc.tensor.matmul(
                            ps1,
                            lhsT=w1_sb[:, ft, k, :],
                            rhs=xg_sb[:, c, k, :],
                            start=(k == 0), stop=(k == KD - 1),
                        )
                    ps3 = ps.tile([P, clen], F32, tag="ps3")
                    for k in range(KD):
                        nc.tensor.matmul(
                            ps3,
                            lhsT=w3_sb[:, ft, k, :],
                            rhs=xg_sb[:, c, k, :],
                            start=(k == 0), stop=(k == KD - 1),
                        )
                    s_sb = hsb.tile([P, clen], F32, tag="s")
                    nc.scalar.activation(s_sb, ps1,
                                         mybir.ActivationFunctionType.Silu)
                    nc.vector.tensor_mul(h_sb[:, ft, :], s_sb, ps3)

                # ---- down projection: y[d, tok] = w2.T h  (gate applied)
                y_sb = ysb.tile([P, KD, clen], F32, tag="y")
                for dt in range(KD):
                    dsl = slice(dt * P, (dt + 1) * P)
                    psy = ps.tile([P, clen], F32, tag="psy")
                    for ft in range(KF):
                        h2, i = divmod(ft, KH)
                        nc.tensor.matmul(
                            psy,
                            lhsT=w2_sb[:, h2, i, dsl],
                            rhs=h_sb[:, ft, :],
                            start=(ft == 0), stop=(ft == KF - 1),
                        )
                    nc.vector.tensor_mul(y_sb[:, dt, :], psy,
                                         gate_sb[:, csl])
                    if c == N_CHUNKS - 1 and dt % 2 == 1:
                        # last chunk: stream the output out per d-tile pair
                        # so only a sliver of DMA remains after the last MM
                        nc.gpsimd.dma_start(
                            y[c].rearrange("p (k j) -> p k j", k=KD)
                            [:, dt - 1:dt + 1, :],
                            y_sb[:, dt - 1:dt + 1, :])
                if c < N_CHUNKS - 1:
                    nc.gpsimd.dma_start(
                        y[c].rearrange("p (k j) -> p k j", k=KD), y_sb)
    nc.compile()
    return nc


USE_RAW_FFN = True


def _build_ffn_raw_nc(c_pad):
    """Raw-Block (no Tile) variant of the FFN kernel: identical dataflow
    and DRAM layouts as _build_ffn_nc, with hand-placed semaphores.
    Avoids the Tile exit-barrier (~9us) and scheduling slack.

    PSUM banks: ps1/ps3/psy each double-buffered by global group parity;
    every consumer increments a counting semaphore that the producer
    checks before reusing the bank.
    """
    nc = bacc.Bacc("TRN2", target_bir_lowering=False, debug=False,
                   num_devices=N_CORES, enable_partition_id=False)
    clen = c_pad // N_CHUNKS
    assert clen * N_CHUNKS == c_pad and clen <= 512

    xg = nc.dram_tensor("xg", [N_CHUNKS, P, KD * clen], BF16,
                        kind="ExternalInput").ap()
    w1 = nc.dram_tensor("w1", [P, KF * KD * P], BF16,
                        kind="ExternalInput").ap()
    w3 = nc.dram_tensor("w3", [P, KF * KD * P], BF16,
                        kind="ExternalInput").ap()
    w2 = nc.dram_tensor("w2", [NH, P, (KF // NH) * D], BF16,
                        kind="ExternalInput").ap()
    gate = nc.dram_tensor("gate", [1, c_pad], F32, kind="ExternalInput").ap()
    y = nc.dram_tensor("y", [N_CHUNKS, P, KD * clen], F32,
                       kind="ExternalOutput").ap()

    KH = KF // NH
    NPC = len(W_PIECES)
    piece_start = []
    s0 = 0
    for npc in W_PIECES:
        piece_start.append(s0)
        s0 += npc
    start_to_piece = {s: i for i, s in enumerate(piece_start)}

    xg_sb = nc.alloc_sbuf_tensor("xg_sb", [P, N_CHUNKS, KD, clen], BF16).ap()
    w1_sb = nc.alloc_sbuf_tensor("w1_sb", [P, KF, KD, P], BF16).ap()
    w3_sb = nc.alloc_sbuf_tensor("w3_sb", [P, KF, KD, P], BF16).ap()
    w2_sb = nc.alloc_sbuf_tensor("w2_sb", [P, NH, KH, D], BF16).ap()
    gate_sb = nc.alloc_sbuf_tensor("gate_sb", [P, c_pad], F32).ap()
    h_sb = nc.alloc_sbuf_tensor("h_sb", [P, N_CHUNKS, KF, clen], BF16).ap()
    s_sb = nc.alloc_sbuf_tensor("s_sb", [P, 2, clen], F32).ap()
    y_sb = nc.alloc_sbuf_tensor("y_sb", [P, N_CHUNKS, KD, clen], F32).ap()
    warm_sb = nc.alloc_sbuf_tensor("warm_sb", [P, 512], BF16).ap()

    # PE phase schedule: software-pipeline chunks so the down-projection
    # of chunk c runs while chunk c+1's activations are already in flight
    # and never stalls on the tail of its own silu/mul chain.
    PHASES = [("u", 0), ("u", 1), ("d", 0), ("u", 2), ("d", 1), ("d", 2)]

    ps1 = nc.alloc_psum_tensor("ps1", [P, 2, 512], F32).ap()
    ps3 = nc.alloc_psum_tensor("ps3", [P, 2, 512], F32).ap()
    psy = nc.alloc_psum_tensor("psy", [P, 2, 512], F32).ap()
    ps_w = nc.alloc_psum_tensor("ps_w", [P, 512], F32).ap()

    w1s = [nc.alloc_semaphore(f"w1s{i}") for i in range(NPC)]
    w3s = [nc.alloc_semaphore(f"w3s{i}") for i in range(NPC)]
    w2s = [nc.alloc_semaphore(f"w2s{i}") for i in range(NH)]
    xgs = [nc.alloc_semaphore(f"xgs{k}") for k in range(KD)]
    xgc = [nc.alloc_semaphore(f"xgc{c}") for c in range(1, N_CHUNKS)]
    gts = nc.alloc_semaphore("gts")
    ps1_done = nc.alloc_semaphore("ps1_done")
    ps3_done = nc.alloc_semaphore("ps3_done")
    silu_done = nc.alloc_semaphore("silu_done")
    hmul_done = nc.alloc_semaphore("hmul_done")
    psy_done = nc.alloc_semaphore("psy_done")
    ymul_done = nc.alloc_semaphore("ymul_done")
    ydma = nc.alloc_semaphore("ydma")

    xg_r = [xg[c].rearrange("p (k j) -> p k j", k=KD)
            for c in range(N_CHUNKS)]
    w1_r = w1.rearrange("p (t k j) -> p t k j", t=KF, k=KD)
    w3_r = w3.rearrange("p (t k j) -> p t k j", t=KF, k=KD)
    y_r = [y[c].rearrange("p (k j) -> p k j", k=KD) for c in range(N_CHUNKS)]
    gate_bcast = bass.AP(tensor=gate.tensor, offset=gate.offset,
                         ap=[[0, P], gate.ap[1]])
    N_YDMA = N_CHUNKS * KD // 2

    with nc.Block(no_gpsimd_drain=True) as block:
        @block.sync
        def _(sync):
            for pi, npc in enumerate(W_PIECES):
                fts = slice(piece_start[pi], piece_start[pi] + npc)
                sync.dma_start(w1_sb[:, fts], w1_r[:, fts]).then_inc(
                    w1s[pi], 16)
            sync.dma_start(
                w2_sb[:, 0], w2[0].rearrange("p (i d) -> p i d", i=KH),
            ).then_inc(w2s[0], 16)
            # last chunk's outputs per d-tile on the (now idle) HWDGE ring
            # so only a sliver of DMA trails the last matmul
            cl = N_CHUNKS - 1
            for dt in range(KD):
                sync.wait_ge(ymul_done, cl * KD + dt + 1)
                sync.dma_start(
                    y_r[cl][:, dt, :], y_sb[:, cl, dt, :],
                ).then_inc(ydma, 16)
            sync.wait_ge(ydma, (KD // 2 * (N_CHUNKS - 1) + KD) * 16)

        @block.gpsimd
        def _(gpsimd):
            gpsimd.memset(warm_sb, 0)
            for k in range(KD):
                gpsimd.dma_start(xg_sb[:, 0, k], xg_r[0][:, k]).then_inc(
                    xgs[k], 16)
            gpsimd.dma_start(gate_sb, gate_bcast).then_inc(gts, 16)
            for c in range(1, N_CHUNKS):
                gpsimd.dma_start(xg_sb[:, c], xg_r[c]).then_inc(
                    xgc[c - 1], 16)
            for c in range(N_CHUNKS - 1):
                for dp in range(KD // 2):
                    gpsimd.wait_ge(ymul_done, c * KD + 2 * dp + 2)
                    gpsimd.dma_start(
                        y_r[c][:, 2 * dp:2 * dp + 2, :],
                        y_sb[:, c, 2 * dp:2 * dp + 2, :],
                    ).then_inc(ydma, 16)

        @block.scalar
        def _(scalar):
            for pi, npc in enumerate(W_PIECES):
                fts = slice(piece_start[pi], piece_start[pi] + npc)
                scalar.dma_start(w3_sb[:, fts], w3_r[:, fts]).then_inc(
                    w3s[pi], 16)
            scalar.dma_start(
                w2_sb[:, 1], w2[1].rearrange("p (i d) -> p i d", i=KH),
            ).then_inc(w2s[1], 16)
            for c in range(N_CHUNKS):
                for ft in range(KF):
                    idx = c * KF + ft
                    scalar.wait_ge(ps1_done, idx + 1)
                    if idx >= 2:
                        # s_sb slot reused two groups later
                        scalar.wait_ge(hmul_done, idx - 1)
                    scalar.activation(
                        s_sb[:, idx % 2, :clen], ps1[:, idx % 2, :clen],
                        mybir.ActivationFunctionType.Silu,
                    ).then_inc(silu_done, 1)

        @block.vector
        def _(vector):
            # program order mirrors the PE phase schedule so neither
            # engine blocks the other through program-order head-of-line
            for (ph, c) in PHASES:
                if ph == "u":
                    for ft in range(KF):
                        idx = c * KF + ft
                        vector.wait_ge(silu_done, idx + 1)
                        vector.wait_ge(ps3_done, idx + 1)
                        vector.tensor_mul(
                            h_sb[:, c, ft, :clen], s_sb[:, idx % 2, :clen],
                            ps3[:, idx % 2, :clen],
                        ).then_inc(hmul_done, 1)
                else:
                    for dt in range(KD):
                        idx2 = c * KD + dt
                        if idx2 == 0:
                            vector.wait_ge(gts, 16)
                        vector.wait_ge(psy_done, idx2 + 1)
                        vector.tensor_mul(
                            y_sb[:, c, dt, :clen], psy[:, idx2 % 2, :clen],
                            gate_sb[:, c * clen:(c + 1) * clen],
                        ).then_inc(ymul_done, 1)

        @block.tensor
        def _(tensor):
            for _i in range(14):
                tensor.matmul(ps_w[:, :clen], lhsT=warm_sb[:, 0:P],
                              rhs=warm_sb[:, :clen], start=True, stop=True)
            for (ph, c) in PHASES:
                if ph == "u":
                    if c >= 1:
                        tensor.wait_ge(xgc[c - 1], 16)
                    for ft in range(KF):
                        idx = c * KF + ft
                        if c == 0 and ft in start_to_piece:
                            tensor.wait_ge(w1s[start_to_piece[ft]], 16)
                            tensor.wait_ge(w3s[start_to_piece[ft]], 16)
                        if idx >= 2:
                            tensor.wait_ge(silu_done, idx - 1)
                        for k in range(KD):
                            if c == 0 and ft == 0:
                                tensor.wait_ge(xgs[k], 16)
                            mm = tensor.matmul(
                                ps1[:, idx % 2, :clen],
                                lhsT=w1_sb[:, ft, k, :],
                                rhs=xg_sb[:, c, k, :],
                                start=(k == 0), stop=(k == KD - 1),
                            )
                            if k == KD - 1:
                                mm.then_inc(ps1_done, 1)
                        if idx >= 2:
                            tensor.wait_ge(hmul_done, idx - 1)
                        for k in range(KD):
                            mm = tensor.matmul(
                                ps3[:, idx % 2, :clen],
                                lhsT=w3_sb[:, ft, k, :],
                                rhs=xg_sb[:, c, k, :],
                                start=(k == 0), stop=(k == KD - 1),
                            )
                            if k == KD - 1:
                                mm.then_inc(ps3_done, 1)
                else:
                    tensor.wait_ge(hmul_done, (c + 1) * KF)
                    if c == 0:
                        tensor.wait_ge(w2s[0], 16)
                        tensor.wait_ge(w2s[1], 16)
                    for dt in range(KD):
                        idx2 = c * KD + dt
                        if idx2 >= 2:
                            tensor.wait_ge(ymul_done, idx2 - 1)
                        for ft in range(KF):
                            h2, i2 = divmod(ft, KH)
                            mm = tensor.matmul(
                                psy[:, idx2 % 2, :clen],
                                lhsT=w2_sb[:, h2, i2, dt * P:(dt + 1) * P],
                                rhs=h_sb[:, c, ft, :clen],
                                start=(ft == 0), stop=(ft == KF - 1),
                            )
                            if ft == KF - 1:
                                mm.then_inc(psy_done, 1)

    nc.compile()
    return nc


def kernel(x, router_w, w1, w2, w3):
    x = np.asarray(x, dtype=np.float32)
    router_w = np.asarray(router_w, dtype=np.float32)
    w1 = np.asarray(w1, dtype=np.float32)
    w2 = np.asarray(w2, dtype=np.float32)
    w3 = np.asarray(w3, dtype=np.float32)

    x_flat = x.reshape(-1, D)
    core_ids = list(range(N_CORES))

    # ---------------- Launch 1: router logits on-device ----------------
    if "nc" not in _router_cache:
        _router_cache["nc"] = _build_router_nc()
    nc_r = _router_cache["nc"]

    rw_pm = _pm(router_w)  # [P, KD*E]
    in_maps = []
    for c in range(N_CORES):
        shard = x_flat[c * SHARD:(c + 1) * SHARD]
        in_maps.append({
            "xT": _pm(np.ascontiguousarray(shard.T)),
            "rw": rw_pm,
        })
    res_r = run_bass_kernel_spmd(nc_r, in_maps, core_ids)
    logits = np.concatenate(
        [res_r.results[c]["logitsT"].T for c in range(N_CORES)], axis=0)

    # ---------------- Host: routing decisions + loss epilogue ----------------
    lmax = logits.max(axis=-1, keepdims=True)
    ex = np.exp(logits - lmax)
    probs = ex / ex.sum(axis=-1, keepdims=True)

    top1 = np.argmax(probs, axis=-1)
    pm_ = probs.copy()
    pm_[np.arange(N_TOK), top1] = -1.0
    top2 = np.argmax(pm_, axis=-1)
    wa = probs[np.arange(N_TOK), top1]
    wb = probs[np.arange(N_TOK), top2]
    den = wa + wb
    g1 = (wa / den).astype(np.float32)
    g2 = (wb / den).astype(np.float32)

    importance = probs.astype(np.float64).mean(axis=0)
    load = np.bincount(top1, minlength=E).astype(np.float64) / N_TOK
    aux_loss = np.float32(E * np.sum(importance * load) * AUX_COEF)
    z_loss = np.float32(np.mean(logits.astype(np.float64) ** 2) * Z_COEF)

    idx_lists, gate_lists = [], []
    for e in range(E):
        sel = np.where((top1 == e) | (top2 == e))[0]
        gates = np.where(top1[sel] == e, g1[sel], g2[sel]).astype(np.float32)
        idx_lists.append(sel)
        gate_lists.append(gates)

    c_max = max(len(s) for s in idx_lists)
    step = 4 * N_CHUNKS
    c_pad = max(384, -(-c_max // step) * step)
    clen = c_pad // N_CHUNKS

    # ---------------- Launch 2: expert-parallel FFN ----------------
    if c_pad not in _ffn_cache:
        build = _build_ffn_raw_nc if USE_RAW_FFN else _build_ffn_nc
        _ffn_cache[c_pad] = build(c_pad)
    nc_f = _ffn_cache[c_pad]

    bf = ml_dtypes.bfloat16
    KH = KF // NH

    in_maps = []
    for e in range(E):
        sel = idx_lists[e]
        # xg[c][p][k*clen+j] = x[sel[c*clen+j], k*128+p]
        xg_full = np.zeros((D, c_pad), dtype=bf)
        xg_full[:, :len(sel)] = x_flat[sel].T.astype(bf)
        xg_t = (xg_full.reshape(KD, P, N_CHUNKS, clen)
                .transpose(2, 1, 0, 3).reshape(N_CHUNKS, P, KD * clen))
        # w1/w3: [p][ft][k][j] f-tile-major
        w1_t = (w1[e].astype(bf).reshape(KD, P, KF, P)
                .transpose(1, 2, 0, 3).reshape(P, KF * KD * P))
        w3_t = (w3[e].astype(bf).reshape(KD, P, KF, P)
                .transpose(1, 2, 0, 3).reshape(P, KF * KD * P))
        w2_t = (w2[e].astype(bf).reshape(NH, KH, P, D)
                .transpose(0, 2, 1, 3).reshape(NH, P, KH * D))
        gt = np.zeros((1, c_pad), dtype=np.float32)
        gt[0, :len(sel)] = gate_lists[e]
        in_maps.append({
            "xg": np.ascontiguousarray(xg_t),
            "w1": np.ascontiguousarray(w1_t),
            "w3": np.ascontiguousarray(w3_t),
            "w2": np.ascontiguousarray(w2_t),
            "gate": gt,
        })
    res_f = run_bass_kernel_spmd(nc_f, in_maps, core_ids)

    global LAST_EXEC_NS
    LAST_RESULTS["router"] = res_r
    LAST_RESULTS["ffn"] = res_f
    if res_r.exec_time_ns is not None or res_f.exec_time_ns is not None:
        LAST_EXEC_NS = (res_r.exec_time_ns or 0) + (res_f.exec_time_ns or 0)

    # ---------------- Host: combine ----------------
    out = np.zeros((N_TOK, D), dtype=np.float32)
    for e in range(E):
        sel = idx_lists[e]
        ye = res_f.results[e]["y"]  # [N_CHUNKS, P, KD*clen]
        # y[c][p][k*clen+j] -> yT [D, c_pad]
        yT = (ye.reshape(N_CHUNKS, P, KD, clen).transpose(2, 1, 0, 3)
              .reshape(D, c_pad))
        out[sel] += yT[:, :len(sel)].T
    return out.reshape(B, S, D), aux_loss, z_loss
